# revision 31
# baseline (speedup 1.0000x reference)
"""Multi-head attention (B=4, S=2048, d_model=1024, H=16) on 8 TRN2 NeuronCores.

Sharding: core c handles batch c//2 and query rows [1024*(c%2), 1024*(c%2)+1024).
Each core redundantly projects K/V for its batch (no collectives) and produces a
disjoint [1024, 1024] slice of the output.

v2 design (ScalarE-exp is the ~285us floor; keep TensorE off the critical path):
  - scores: per head pair, the two heads' K=64 matmuls sit in disjoint PE row
    groups (partitions 0-63 / 64-127) and are emitted adjacently so the PE
    dual-issues them -> 2 concurrent N=512 streams = full array.
  - attnV: col-tiled 2x via tile_position (0,0)/(0,64): head0 -> PSUM rows
    0-63, head1 -> rows 64-127 of ONE bank, M=64 each, concurrent streams.
  - softmax denominators: DVE elementwise partial sums of the exp tiles, then
    one gpsimd partition_all_reduce per (g,qb) block; reciprocal + normalize on
    DVE straight out of PSUM (no DRAM bounce, no broadcast matmul).
  - all input DMA spread across 4 engine queues up front; V/K/Q projections and
    the out-projection interleave into the ScalarE-bound attention phase.
"""

import numpy as np

import bass_rust
import concourse.bass as bass
import concourse.mybir as mybir
import concourse.tile as tile
from concourse.bass_utils import run_bass_kernel_spmd
from concourse.vector_clock import ScopedClock

F32 = mybir.dt.float32
F32R = mybir.dt.float32r
BF16 = mybir.dt.bfloat16
AF = mybir.ActivationFunctionType
ADD = mybir.AluOpType.add
MULT = mybir.AluOpType.mult

D_MODEL = 1024
B = 4
S = 2048
N_CORES = 8
QL = 1024  # query rows per core
NPAIR = 8  # head pairs
NK = D_MODEL // 128  # contraction chunks
NT = S // 128  # key chunks

# ---------------------------------------------------------------------------
# Workaround for this container's walrus build: each instruction may carry at
# most ONE embedded sync-wait ("Too many sync wait commands" otherwise). Tile
# attaches several; split the extras onto same-engine NOPs placed immediately
# before the instruction (engine queues are in-order => identical semantics).
_MAX_WAITS = 1


def _patched_lower(self, ordered):
    nc = self.nc
    for bb_name, insts in ordered.items():
        new_list = []
        for inst in insts:
            si = inst.sync_info
            waits = list(si.on_wait) if si is not None and si.on_wait else []
            if len(waits) > _MAX_WAITS:
                updates = list(si.on_update) if si.on_update else []
                for w in waits[:-_MAX_WAITS]:
                    nop = bass_rust.InstNoOp(
                        name=nc.get_next_instruction_name(),
                        engine=inst.engine,
                        debug=inst.debug,
                        sync_info=bass_rust.SyncInfo(on_wait=[w], on_update=[]),
                    )
                    new_list.append(nop)
                inst.sync_info = bass_rust.SyncInfo(
                    on_wait=waits[-_MAX_WAITS:], on_update=updates
                )
            new_list.append(inst)
        insts[:] = new_list
    return tile.TileContext._orig_lower_ordered_insts(self, ordered)


def _patched_drain(self, tick_clock, wait_clock):
    probe = self.nc.sync.nop(nofuse=True)
    wait_clock.add_sem_waits(probe.ins, ScopedClock({None: tick_clock.global_clock}))
    si = probe.ins.sync_info
    waits = list(si.on_wait) if si is not None and si.on_wait else []
    if len(waits) > _MAX_WAITS:
        probe.ins.sync_info = bass_rust.SyncInfo(
            on_wait=waits[:_MAX_WAITS], on_update=[]
        )
        for w in waits[_MAX_WAITS:]:
            n = self.nc.sync.nop(nofuse=True)
            n.ins.sync_info = bass_rust.SyncInfo(on_wait=[w], on_update=[])
    self.nc.sync.drain()
    self.nc.all_engine_barrier()
    assert self.sems is not None
    popped = self.nc._tile_sem_poison_stack.pop()
    assert popped is self._sem_poison
    self.nc.clear_and_free_semaphores(list(self.sems.allocated().values()))
    self.nc.all_engine_barrier()


import concourse.bass_utils as _bu


def _install_patch():
    if not hasattr(tile.TileContext, "_orig_lower_ordered_insts"):
        tile.TileContext._orig_lower_ordered_insts = (
            tile.TileContext._lower_ordered_insts
        )
        tile.TileContext._lower_ordered_insts = _patched_lower
        tile.TileContext._drain_and_barrier = _patched_drain


# ---------------------------------------------------------------------------


def _build_bass():
    nc = bass.Bass()
    qt = nc.dram_tensor("qt", [D_MODEL, QL], BF16, kind="ExternalInput")
    kt = nc.dram_tensor("kt", [D_MODEL, S], BF16, kind="ExternalInput")
    vt = nc.dram_tensor("vt", [NT, 128, 1024], BF16, kind="ExternalInput")
    wq = nc.dram_tensor("wq", [NPAIR, NK, 128, 128], BF16, kind="ExternalInput")
    wk = nc.dram_tensor("wk", [NPAIR, NK, 128, 128], BF16, kind="ExternalInput")
    wv = nc.dram_tensor("wv", [D_MODEL, D_MODEL], BF16, kind="ExternalInput")
    wo = nc.dram_tensor("wo", [D_MODEL, D_MODEL], BF16, kind="ExternalInput")
    bqt = nc.dram_tensor("bqt", [128, NK], F32, kind="ExternalInput")
    bkt = nc.dram_tensor("bkt", [128, NK], F32, kind="ExternalInput")
    bvr = nc.dram_tensor("bvr", [128, D_MODEL], F32, kind="ExternalInput")
    bor = nc.dram_tensor("bor", [128, D_MODEL], F32, kind="ExternalInput")
    ones2 = nc.dram_tensor("ones2", [128, 128], BF16, kind="ExternalInput")
    onec = nc.dram_tensor("onec", [128, 1], BF16, kind="ExternalInput")
    out = nc.dram_tensor("out", [QL, D_MODEL], F32, kind="ExternalOutput")

    with tile.TileContext(nc) as tc:
        _emit(nc, tc, locals())
    return nc


def _emit(nc, tc, t):
    qt, kt, vt = t["qt"], t["kt"], t["vt"]
    wq, wk, wv, wo = t["wq"], t["wk"], t["wv"], t["wo"]
    bqt, bkt, bvr, bor = t["bqt"], t["bkt"], t["bvr"], t["bor"]
    out = t["out"]
    ones2, onec = t["ones2"], t["onec"]

    P = tc.tile_pool

    with (
        P(name="consts", bufs=1) as consts,
        P(name="pkv", bufs=1) as pkv,
        P(name="pv", bufs=1) as pv,
        P(name="pxn", bufs=1) as pxn,
        P(name="pwo", bufs=1) as pwo,
        P(name="expp", bufs=3) as expp,
        P(name="pden", bufs=2) as pden,
        P(name="prec", bufs=1) as prec,
        P(name="stg", bufs=2) as stg,
        P(name="psS", bufs=2, space="PSUM") as psS,
        P(name="psacc", bufs=2, space="PSUM") as psacc,
        P(name="psP", bufs=2, space="PSUM") as psP,
    ):
        # ---- constants --------------------------------------------------
        bqt_t = consts.tile([128, NK], F32, tag="bqt")
        nc.sync.dma_start(bqt_t[:], bqt[:])
        bkt_t = consts.tile([128, NK], F32, tag="bkt")
        nc.sync.dma_start(bkt_t[:], bkt[:])
        bvr_t = consts.tile([128, D_MODEL], F32, tag="bvr")
        nc.sync.dma_start(bvr_t[:], bvr[:])
        bor_t = consts.tile([128, D_MODEL], F32, tag="bor")
        nc.sync.dma_start(bor_t[:], bor[:])
        ones_t = consts.tile([128, 128], BF16, tag="ones2")
        nc.sync.dma_start(ones_t[:], ones2[:])
        onec_t = consts.tile([128, 1], BF16, tag="onec")
        nc.sync.dma_start(onec_t[:], onec[:])

        # ---- long-lived result tiles ------------------------------------
        KT = [
            pkv.tile([128, S], BF16, name=f"ktg{g}", tag=f"ktg{g}")
            for g in range(NPAIR)
        ]
        QT = [
            pkv.tile([128, QL], BF16, name=f"qtg{g}", tag=f"qtg{g}")
            for g in range(NPAIR)
        ]
        v_tiles = [
            pv.tile([128, 1024], BF16, name=f"v{c}", tag=f"v{c}") for c in range(NT)
        ]
        xn_tiles = [
            pxn.tile([128, QL], BF16, name=f"xn{g}", tag=f"xn{g}")
            for g in range(NPAIR)
        ]

        def emit_block(g, qb, fill, ensure_v):
            """One attention block: head pair g, query slice qb (512 queries).
            fill() emits filler units (projection j-groups) at a fixed cadence;
            ensure_v(c) guarantees V chunk c is emitted before use."""
            q0 = 512 * qb

            def emit_scores(cg):
                scs = []
                for h in range(2):
                    sch = psS.tile([128, QL], F32, name=f"sc{h}", tag="scores")
                    scs.append(sch)
                for ci in range(2):
                    c = 2 * cg + ci
                    for h in range(2):
                        nc.tensor.matmul(
                            scs[h][:, 512 * ci : 512 * ci + 512],
                            KT[g][64 * h : 64 * h + 64, 128 * c : 128 * c + 128],
                            QT[g][64 * h : 64 * h + 64, q0 : q0 + 512],
                            start=True,
                            stop=True,
                            skip_group_check=True,
                            tile_position=(64 * h, 0),
                        )
                return scs

            acc = psacc.tile([128, 512], F32, tag="acc")
            den = pden.tile([128, QL], F32, tag="den")
            denb = pden.tile([128, QL], BF16, tag="denb")
            sc_cur = emit_scores(0)
            for cg in range(NT // 2):
                exs = []
                for h in range(2):
                    ex = expp.tile([128, QL], BF16, tag="exp")
                    nc.scalar.activation(ex[:], sc_cur[h][:], AF.Exp, scale=0.125)
                    exs.append(ex)
                # attnV: both heads col-tiled into one PSUM bank, concurrent
                ensure_v(2 * cg + 1)
                for ci in range(2):
                    c = 2 * cg + ci
                    nc.tensor.matmul(
                        acc[0:64, :],
                        v_tiles[c][:, 128 * g : 128 * g + 64],
                        exs[0][:, 512 * ci : 512 * ci + 512],
                        start=(c == 0),
                        stop=(c == NT - 1),
                        skip_group_check=True,
                        tile_position=(0, 0),
                    )
                    nc.tensor.matmul(
                        acc[64:128, :],
                        v_tiles[c][:, 128 * g + 64 : 128 * g + 128],
                        exs[1][:, 512 * ci : 512 * ci + 512],
                        start=(c == 0),
                        stop=(c == NT - 1),
                        skip_group_check=True,
                        tile_position=(0, 64),
                    )
                fill()
                # denominator partial sums (keys collapse to 128 partitions);
                # the last add casts to bf16 for the fp-matched ones-matmul
                for h in range(2):
                    if cg == 0:
                        nc.vector.tensor_tensor(
                            den[:, 512 * h : 512 * h + 512],
                            exs[h][:, 0:512],
                            exs[h][:, 512:1024],
                            ADD,
                        )
                    else:
                        last = cg == NT // 2 - 1
                        nc.vector.tensor_tensor(
                            den[:, 512 * h : 512 * h + 512],
                            den[:, 512 * h : 512 * h + 512],
                            exs[h][:, 0:512],
                            ADD,
                        )
                        nc.vector.tensor_tensor(
                            (denb if last else den)[:, 512 * h : 512 * h + 512],
                            den[:, 512 * h : 512 * h + 512],
                            exs[h][:, 512:1024],
                            ADD,
                        )
                if cg + 1 < NT // 2:
                    sc_cur = emit_scores(cg + 1)
                else:
                    sc_cur = None

            # finalize: reduce denominators across partitions (col-tiled ones
            # matmuls -> PSUM rows 0/64), reciprocal, broadcast (K=2 matmul),
            # then normalize acc straight out of PSUM.
            denp = psP.tile([128, 512], F32, tag="pp")
            for h in range(2):
                nc.tensor.matmul(
                    denp[64 * h : 64 * h + 1, :],
                    onec_t[:, 0:1],
                    denb[:, 512 * h : 512 * h + 512],
                    start=True,
                    stop=True,
                    skip_group_check=True,
                    tile_position=(0, 64 * h),
                )
            srr = prec.tile([65, 512], BF16, tag="srr")
            with nc.allow_low_precision(reason="bf16 softmax recip, tol 2e-2"):
                nc.vector.reciprocal(srr[0:1, :], denp[0:1, :])
                nc.vector.reciprocal(srr[64:65, :], denp[64:65, :])
            rep = psP.tile([128, 512], F32, tag="pp")
            nc.tensor.matmul(
                rep[:],
                ones_t[0:1, :],
                srr[0:1, :],
                start=True,
                stop=False,
                skip_group_check=True,
            )
            nc.tensor.matmul(
                rep[:],
                ones_t[64:65, :],
                srr[64:65, :],
                start=False,
                stop=True,
                skip_group_check=True,
            )
            repc = prec.tile([128, 512], BF16, tag="repc")
            nc.vector.tensor_copy(repc[:], rep[:])
            nc.vector.tensor_tensor(
                xn_tiles[g][0:64, q0 : q0 + 512], acc[0:64, :], repc[0:64, :], MULT
            )
            nc.vector.tensor_tensor(
                xn_tiles[g][64:128, q0 : q0 + 512], acc[64:128, :], repc[64:128, :],
                MULT,
            )

        # ---- staged pools: raw K/Q + weights live only until projections done
        with (
            P(name="kstr", bufs=1) as kstr,
            P(name="qstr", bufs=1) as qstr,
            P(name="wvp", bufs=1) as wvp,
            P(name="vstr", bufs=2) as vstr,
            P(name="wks", bufs=2) as wks,
            P(name="wqs", bufs=2) as wqs,
        ):
            # -------- input DMAs spread over 4 engine queues --------------
            kfull = []
            for k in range(NK):
                ktile = kstr.tile([128, S], BF16, name=f"ktf{k}", tag=f"ktf{k}")
                nc.scalar.dma_start(ktile[:], kt[128 * k : 128 * k + 128, :])
                kfull.append(ktile)
            qfull = []
            for k in range(NK):
                qtile = qstr.tile([128, QL], BF16, name=f"qtf{k}", tag=f"qtf{k}")
                nc.scalar.dma_start(qtile[:], qt[128 * k : 128 * k + 128, :])
                qfull.append(qtile)
            wv_tiles = []
            for k in range(NK):
                wvt = wvp.tile([128, D_MODEL], BF16, name=f"wv{k}", tag=f"wv{k}")
                nc.gpsimd.dma_start(wvt[:], wv[128 * k : 128 * k + 128, :])
                wv_tiles.append(wvt)

            # -------- filler unit emitters --------------------------------
            def emit_vchunk(c):
                vts = vstr.tile([128, 1024], BF16, tag="vts")
                nc.sync.dma_start(vts[:], vt[c])
                for j in range(2):
                    ps = psP.tile([128, 512], F32, tag="pp")
                    for k in range(NK):
                        nc.tensor.matmul(
                            ps[:],
                            vts[:, 128 * k : 128 * k + 128],
                            wv_tiles[k][:, 512 * j : 512 * j + 512],
                            start=(k == 0),
                            stop=(k == NK - 1),
                            skip_group_check=True,
                        )
                    nc.vector.tensor_tensor(
                        v_tiles[c][:, 512 * j : 512 * j + 512],
                        ps[:],
                        bvr_t[:, 512 * j : 512 * j + 512],
                        ADD,
                    )

            def emit_kproj(g, half):
                wkg = []
                for k in range(NK):
                    wkt = wks.tile([128, 128], BF16, tag=f"wks{k}")
                    nc.gpsimd.dma_start(wkt[:], wk[g, k])
                    wkg.append(wkt)
                for j in range(2):
                    ps = psP.tile([128, 512], F32, tag="pp")
                    for k in range(NK):
                        nc.tensor.matmul(
                            ps[:],
                            wkg[k][:],
                            kfull[k][
                                :,
                                1024 * half + 512 * j : 1024 * half + 512 * j + 512,
                            ],
                            start=(k == 0),
                            stop=(k == NK - 1),
                            skip_group_check=True,
                        )
                    nc.vector.tensor_scalar_add(
                        KT[g][:, 1024 * half + 512 * j : 1024 * half + 512 * j + 512],
                        ps[:],
                        bkt_t[:, g : g + 1],
                    )

            def emit_qproj(g):
                wqg = []
                for k in range(NK):
                    wqt = wqs.tile([128, 128], BF16, tag=f"wqs{k}")
                    nc.gpsimd.dma_start(wqt[:], wq[g, k])
                    wqg.append(wqt)
                for j in range(2):
                    ps = psP.tile([128, 512], F32, tag="pp")
                    for k in range(NK):
                        nc.tensor.matmul(
                            ps[:],
                            wqg[k][:],
                            qfull[k][:, 512 * j : 512 * j + 512],
                            start=(k == 0),
                            stop=(k == NK - 1),
                            skip_group_check=True,
                        )
                    nc.vector.tensor_scalar_add(
                        QT[g][:, 512 * j : 512 * j + 512], ps[:], bqt_t[:, g : g + 1]
                    )

            # -------- prologue: V chunks 0-5 + K/Q proj for g=0,1 ---------
            for c in range(4):
                emit_vchunk(c)
            emit_kproj(0, 0)
            emit_kproj(0, 1)
            emit_qproj(0)
            emit_vchunk(4)
            emit_vchunk(5)
            emit_kproj(1, 0)
            emit_kproj(1, 1)
            emit_qproj(1)

            # -------- filler queue: remaining V chunks, then K/Q g=2..7 ---
            fillers = []
            for c in range(6, NT):
                fillers.append(("v", c, emit_vchunk, (c,)))
            for g in range(2, NPAIR):
                fillers.append(("k", -1, emit_kproj, (g, 0)))
                fillers.append(("k", -1, emit_kproj, (g, 1)))
                fillers.append(("q", -1, emit_qproj, (g,)))
            state = {"idx": 0, "budget": 0.0, "v_done": 5}

            def _pop():
                if state["idx"] >= len(fillers):
                    return False
                kind, c, fn, args = fillers[state["idx"]]
                state["idx"] += 1
                fn(*args)
                if kind == "v":
                    state["v_done"] = c
                return True

            def fill():
                # consume filler units at a fixed cadence: ~0.45 unit per cg
                state["budget"] += 0.45
                while state["budget"] >= 1.0:
                    state["budget"] -= 1.0
                    if not _pop():
                        return

            def ensure_v(c):
                while state["v_done"] < c:
                    if not _pop():
                        raise RuntimeError("V chunk queue exhausted early")

            # blocks g=0..5 inside the staged-pool scope
            for g in range(6):
                for qb in range(2):
                    emit_block(g, qb, fill, ensure_v)
            while _pop():
                pass

        # blocks g=6,7 + out-projection (staging pools closed, SBUF freed)
        with P(name="pwo", bufs=1) as pwo:
            wo_tiles = [
                pwo.tile([128, D_MODEL], BF16, name=f"wo{k}", tag=f"wo{k}")
                for k in range(NK)
            ]
            for k in range(NK):
                nc.gpsimd.dma_start(wo_tiles[k][:], wo[128 * k : 128 * k + 128, :])

            def nofill():
                return

            def noensure(c):
                return

            for gq in [(6, 0), (6, 1), (7, 0), (7, 1)]:
                emit_block(gq[0], gq[1], nofill, noensure)

            for m in range(QL // 128):
                for j in range(2):
                    ps = psP.tile([128, 512], F32, tag="pp")
                    for g in range(NPAIR):
                        nc.tensor.matmul(
                            ps[:],
                            xn_tiles[g][:, 128 * m : 128 * m + 128],
                            wo_tiles[g][:, 512 * j : 512 * j + 512],
                            start=(g == 0),
                            stop=(g == NPAIR - 1),
                            skip_group_check=True,
                        )
                    ot = stg.tile([128, 512], F32, tag="outs")
                    nc.vector.tensor_tensor(
                        ot[:], ps[:], bor_t[:, 512 * j : 512 * j + 512], ADD
                    )
                    nc.sync.dma_start(
                        out[128 * m : 128 * m + 128, 512 * j : 512 * j + 512], ot[:]
                    )


_NC_CACHE = None
LAST_RESULT = None


def _get_nc():
    global _NC_CACHE
    if _NC_CACHE is None:
        _install_patch()
        _NC_CACHE = _build_bass()
    return _NC_CACHE


def kernel(q, k, v, w_q, b_q, w_k, b_k, w_v, b_v, w_o, b_o):
    global LAST_RESULT
    import ml_dtypes

    q = np.asarray(q, np.float32)
    k = np.asarray(k, np.float32)
    v = np.asarray(v, np.float32)

    def _tile_w(w):
        # [in, out] -> [g, k, 128, 128] contiguous tiles: w[128k:+128, 128g:+128]
        return np.ascontiguousarray(
            np.asarray(w, np.float32)
            .reshape(NK, 128, NPAIR, 128)
            .transpose(2, 0, 1, 3)
        ).astype(ml_dtypes.bfloat16)

    w_q = _tile_w(w_q)
    w_k = _tile_w(w_k)
    w_v = np.asarray(w_v, np.float32).astype(ml_dtypes.bfloat16)
    w_o = np.asarray(w_o, np.float32).astype(ml_dtypes.bfloat16)
    b_q = np.asarray(b_q, np.float32)
    b_k = np.asarray(b_k, np.float32)
    b_v = np.asarray(b_v, np.float32)
    b_o = np.asarray(b_o, np.float32)

    bqt = np.ascontiguousarray(b_q.reshape(NK, 128).T)
    bkt = np.ascontiguousarray(b_k.reshape(NK, 128).T)
    bvr = np.ascontiguousarray(np.broadcast_to(b_v[None, :], (128, D_MODEL)))
    bor = np.ascontiguousarray(np.broadcast_to(b_o[None, :], (128, D_MODEL)))
    ones2 = np.zeros((128, 128), ml_dtypes.bfloat16)
    ones2[0, 0:64] = 1.0
    ones2[64, 64:128] = 1.0
    onec = np.ones((128, 1), ml_dtypes.bfloat16)

    in_maps = []
    for c in range(N_CORES):
        b = c // 2
        r0 = QL * (c % 2)
        in_maps.append(
            {
                "qt": np.ascontiguousarray(q[b, r0 : r0 + QL, :].T).astype(
                    ml_dtypes.bfloat16
                ),
                "kt": np.ascontiguousarray(k[b].T).astype(ml_dtypes.bfloat16),
                "vt": np.ascontiguousarray(
                    v[b]
                    .T.reshape(8, 128, 16, 128)
                    .transpose(2, 1, 0, 3)
                    .reshape(16, 128, 1024)
                ).astype(ml_dtypes.bfloat16),
                "wq": w_q,
                "wk": w_k,
                "wv": w_v,
                "wo": w_o,
                "bqt": bqt,
                "bkt": bkt,
                "bvr": bvr,
                "bor": bor,
                "ones2": ones2,
                "onec": onec,
            }
        )

    nc = _get_nc()
    res = run_bass_kernel_spmd(nc, in_maps, list(range(N_CORES)))
    LAST_RESULT = res

    outp = np.empty((B, S, D_MODEL), np.float32)
    for c in range(N_CORES):
        b = c // 2
        r0 = QL * (c % 2)
        outp[b, r0 : r0 + QL, :] = res.results[c]["out"]
    return outp
